# revision 18
# baseline (speedup 1.0000x reference)
"""8-core Trainium2 Bass kernel for a 2-layer GCN + mean-pool + 4-layer MLP.

Strategy (graph/data parallel, per the sharding hint):
  - Nodes are partitioned into 8 contiguous ranges of 6250 (core c owns
    [c*6250, (c+1)*6250)).  Edges are bucketed by dst-owner.
  - Message tables (h @ W) * src_isqrt are built shard-wise (TensorE) and
    replicated with an AllGather.  Per-edge message rows are fetched from
    the table with indirect-DMA gathers (OPI rows x 512B per op) and
    accumulated into a per-node DRAM accumulator with indirect-DMA
    scatter-adds keyed by local dst (f32).  Each dst's edges are spread
    round-robin across ops on the host so no scatter op repeats a dst row
    (concurrent RMW on the same row within one op loses updates on HW).
    Normalization (D^-1/2 A D^-1/2) uses host-precomputed 1/sqrt(deg).
  - Per-graph pooled sums+counts [64,129] are AllReduce'd, and the small MLP
    runs replicated on every core.

  All per-core inputs are packed into a single uint8 blob (x quantized to
  int8, edge index streams int16) to minimize host->device transfer cost;
  the blob is sliced/bitcast on device.  Pad slots gather table row 0 and
  scatter into dump rows >= 6250 of the padded accumulator, keeping every
  DMA op full-size and static.
"""

import sys

import numpy as np

sys.path.insert(0, "/opt/trn_rl_repo")

import jax

# Persistent XLA compilation cache: the axon shim in bass_utils rebuilds the
# jit wrapper on every run_bass_kernel_spmd call, which re-invokes the full
# BIR->NEFF backend compile (~1.2s) unless the executable cache can serve it.
try:
    jax.config.update("jax_compilation_cache_dir", "/tmp/jax_cc_cache")
    jax.config.update("jax_persistent_cache_min_compile_time_secs", 0)
    jax.config.update("jax_persistent_cache_min_entry_size_bytes", -1)
except Exception:
    pass

import ml_dtypes

BF16 = ml_dtypes.bfloat16

N = 50000
E = 1600000
D = 128
G = 64
C = 8
NS = N // C            # 6250 nodes per core
P = 128
NT = (NS + P - 1) // P  # 49 node windows per core
NSP = NT * P            # 6272 (padded rows; >= NS are dump rows)
HALF = 25088


# ---------------------------------------------------------------------------
# Blob layout (shared between host packing and device program)
# ---------------------------------------------------------------------------

def _np_dt(mdt_name):
    return {
        "int8": np.int8, "uint8": np.uint8, "int16": np.int16,
        "bfloat16": BF16, "float32": np.float32,
    }[mdt_name]


def _blob_layout(CL, CHH, OPI):
    CT = CL + CHH
    specs = [
        ("xq", P, NSP, "int8"),
        ("esrc", 16, CT * (OPI // 16), "int16"),
        ("edst", 16, CT * (OPI // 16), "int16"),
        ("gid", P, NT, "int8"),
        ("sisq", P, NT, "float32"),
        ("disq", P, NT, "float32"),
        ("W1", D, D, "bfloat16"),
        ("W2", D, D, "bfloat16"),
        ("xscale", P, 1, "float32"),
        ("b1", 1, D, "float32"),
        ("b2", 1, D, "float32"),
        ("Wc1", D, 64, "float32"),
        ("Wc2", 64, 32, "float32"),
        ("Wc3", 32, 16, "float32"),
        ("Wc4", 16, 1, "float32"),
        ("bc1", 64, 1, "float32"),
        ("bc2", 32, 1, "float32"),
        ("bc3", 16, 1, "float32"),
        ("bc4", 1, 1, "float32"),
    ]
    layout = {}
    off = 0
    for name, r, c, dtn in specs:
        nb = r * c * np.dtype(_np_dt(dtn)).itemsize
        layout[name] = (off, r, c, dtn, nb)
        off += (nb + 511) // 512 * 512
    total = (off + 511) // 512 * 512
    return layout, total


# ---------------------------------------------------------------------------
# Host-side sharding prep
# ---------------------------------------------------------------------------

def _wrap16(vals):
    """vals [k*16] -> [16, k] wrapped (idx j at (j%16, j//16))."""
    return np.ascontiguousarray(vals.reshape(-1, 16).T)


def _rr_assign(d_local, nops):
    """Round-robin edges of each dst across ops so no op repeats a dst row
    (avoids duplicate-row RMW races within one scatter op).

    Returns (op, order) where edges (reordered by `order`) are grouped by
    op with per-op unique dst."""
    order = np.argsort(d_local, kind="stable")
    ds = d_local[order]
    first = np.searchsorted(ds, ds)
    rank = np.arange(ds.size) - first
    h = (ds.astype(np.int64) * 2654435761) % nops
    op = (h + rank) % nops
    order2 = np.argsort(op, kind="stable")
    return op[order2], order[order2]


def _prep_blobs(x, src, dst, graph_id):
    src = np.asarray(src).astype(np.int64)
    dst = np.asarray(dst).astype(np.int64)
    x = np.asarray(x).astype(np.float32)
    graph_id = np.asarray(graph_id).astype(np.int64)

    out_deg = np.maximum(np.bincount(src, minlength=N), 1).astype(np.float64)
    in_deg = np.maximum(np.bincount(dst, minlength=N), 1).astype(np.float64)
    s_isq = (1.0 / np.sqrt(out_deg)).astype(np.float32)
    d_isq = (1.0 / np.sqrt(in_deg)).astype(np.float32)

    xscale = np.float32(np.abs(x).max() / 127.0)
    xq = np.clip(np.round(x / xscale), -127, 127).astype(np.int8)

    dst_owner = dst // NS
    lo_mask = src < HALF
    per_core = []
    CL = 0
    CHH = 0
    maxload = 0
    for c in range(C):
        me = dst_owner == c
        mlo = me & lo_mask
        mhi = me & ~lo_mask
        nlo = int(np.count_nonzero(mlo))
        nhi = int(np.count_nonzero(mhi))
        degl = int(np.bincount(dst[mlo] - c * NS, minlength=NS).max())
        degh = int(np.bincount(dst[mhi] - c * NS, minlength=NS).max())
        CL = max(CL, (nlo + 2047) // 2048, degl + 1)
        CHH = max(CHH, (nhi + 2047) // 2048, degh + 1)
        per_core.append((mlo, mhi))

    # per-op load with round-robin assignment; OPI = padded op capacity
    assigns = []
    for c in range(C):
        mlo, mhi = per_core[c]
        a_lo = _rr_assign((dst[mlo] - c * NS).astype(np.int64), CL)
        a_hi = _rr_assign((dst[mhi] - c * NS).astype(np.int64), CHH)
        for op, _ in (a_lo, a_hi):
            if op.size:
                maxload = max(maxload, int(np.bincount(op).max()))
        assigns.append((a_lo, a_hi))
    OPI = (maxload + 127) // 128 * 128

    shards = []
    for c in range(C):
        mlo, mhi = per_core[c]
        (op_lo, ord_lo), (op_hi, ord_hi) = assigns[c]
        parts_e = []
        parts_d = []
        for mask, nops, op, order, base in [
            (mlo, CL, op_lo, ord_lo, 0),
            (mhi, CHH, op_hi, ord_hi, HALF),
        ]:
            s_adj = (src[mask] - base).astype(np.int16)[order]
            d_loc = (dst[mask] - c * NS).astype(np.int16)[order]
            es = np.zeros(nops * OPI, dtype=np.int16)
            ds_ = np.full(nops * OPI, NS, dtype=np.int16)  # pad -> dump row
            pos = np.arange(op.size) - np.searchsorted(op, op)
            slot = op * OPI + pos
            assert pos.max(initial=0) < OPI
            es[slot] = s_adj
            ds_[slot] = d_loc
            parts_e.append(_wrap16(es))
            parts_d.append(_wrap16(ds_))
        esrc = np.concatenate(parts_e, axis=1)
        edst = np.concatenate(parts_d, axis=1)

        xqT = np.zeros((P, NSP), dtype=np.int8)
        xqT[:, :NS] = xq[c * NS:(c + 1) * NS].T
        gid = np.full((P, NT), -1, dtype=np.int8)
        gid.T.flat[:NS] = graph_id[c * NS:(c + 1) * NS].astype(np.int8)
        sisq = np.ones((P, NT), dtype=np.float32)
        sisq.T.flat[:NS] = s_isq[c * NS:(c + 1) * NS]
        disq = np.ones((P, NT), dtype=np.float32)
        disq.T.flat[:NS] = d_isq[c * NS:(c + 1) * NS]
        shards.append(dict(esrc=esrc, edst=edst, xq=xqT, gid=gid,
                           sisq=sisq, disq=disq))
    return shards, xscale, CL, CHH, OPI


def _pack_blobs(shards, xscale, CL, CHH, OPI, W1, b1, W2, b2,
                Wc1, bc1, Wc2, bc2, Wc3, bc3, Wc4, bc4):
    layout, total = _blob_layout(CL, CHH, OPI)
    common = dict(
        W1=np.asarray(W1).astype(BF16),
        W2=np.asarray(W2).astype(BF16),
        xscale=np.full((P, 1), xscale, dtype=np.float32),
        b1=np.asarray(b1, dtype=np.float32).reshape(1, D),
        b2=np.asarray(b2, dtype=np.float32).reshape(1, D),
        Wc1=np.asarray(Wc1, dtype=np.float32),
        Wc2=np.asarray(Wc2, dtype=np.float32),
        Wc3=np.asarray(Wc3, dtype=np.float32),
        Wc4=np.asarray(Wc4, dtype=np.float32),
        bc1=np.asarray(bc1, dtype=np.float32).reshape(64, 1),
        bc2=np.asarray(bc2, dtype=np.float32).reshape(32, 1),
        bc3=np.asarray(bc3, dtype=np.float32).reshape(16, 1),
        bc4=np.asarray(bc4, dtype=np.float32).reshape(1, 1),
    )
    blobs = []
    for sh in shards:
        blob = np.zeros((1, total), dtype=np.uint8)
        for name, (off, r, c, dtn, nb) in layout.items():
            arr = sh[name] if name in sh else common[name]
            a = np.ascontiguousarray(arr, dtype=_np_dt(dtn))
            assert a.shape == (r, c), (name, a.shape, (r, c))
            blob[0, off:off + nb] = a.view(np.uint8).reshape(-1)
        blobs.append(blob)
    return blobs, total


# ---------------------------------------------------------------------------
# Bass program
# ---------------------------------------------------------------------------

_PROGRAM_CACHE = {}


def _build_program(CL, CHH, OPI):
    CT = CL + CHH
    ICOL = OPI // 16             # idx cols per op
    TPC = OPI // P               # msg tiles per op
    layout, NBYTES = _blob_layout(CL, CHH, OPI)
    import concourse.bacc as bacc
    import concourse.mybir as mybir
    import concourse.tile as tile

    f32 = mybir.dt.float32
    bf16 = mybir.dt.bfloat16
    i8 = mybir.dt.int8
    i16 = mybir.dt.int16
    Alu = mybir.AluOpType
    Act = mybir.ActivationFunctionType
    MDT = {"int8": i8, "uint8": mybir.dt.uint8, "int16": i16,
           "bfloat16": bf16, "float32": f32}

    nc = bacc.Bacc("TRN2", target_bir_lowering=False, debug=False,
                   num_devices=C)

    t_blob = nc.dram_tensor("blob", [1, NBYTES], mybir.dt.uint8,
                            kind="ExternalInput")
    t_out = nc.dram_tensor("out", [1, G], f32, kind="ExternalOutput")

    def bview(name):
        off, r, c, dtn, nb = layout[name]
        return (t_blob[0:1, off:off + nb].bitcast(MDT[dtn])
                .rearrange("a (p c) -> (a p) c", p=r))

    rg = [list(range(C))]

    with tile.TileContext(nc) as tc:
        with (
            tc.tile_pool(name="const", bufs=1) as cp,
            tc.tile_pool(name="dram", bufs=1, space="DRAM") as dp,
            tc.tile_pool(name="gbuf", bufs=3) as gp,
            tc.tile_pool(name="tmp", bufs=6) as tp,
        ):
            # ---- persistent SBUF tensors ----
            esrc_sb = cp.tile([P, CT * ICOL], i16)
            edst_sb = cp.tile([P, CT * ICOL], i16)
            gid8_sb = cp.tile([P, NT], i8)
            gid_sb = cp.tile([P, NT], bf16)
            xq_sb = cp.tile([P, NSP], i8)
            xT_sb = cp.tile([P, NSP], bf16)
            sisq_sb = cp.tile([P, NT], f32)
            disq_sb = cp.tile([P, NT], f32)
            W1_sb = cp.tile([D, D], bf16)
            W2_sb = cp.tile([D, D], bf16)
            xscale_sb = cp.tile([P, 1], f32)
            b1row_sb = cp.tile([1, D], f32)
            b2row_sb = cp.tile([1, D], f32)
            b1r_sb = cp.tile([P, D], f32)
            b2r_sb = cp.tile([P, D], f32)
            Wc1_sb = cp.tile([D, 64], f32)
            Wc2_sb = cp.tile([64, 32], f32)
            Wc3_sb = cp.tile([32, 16], f32)
            Wc4_sb = cp.tile([16, 1], f32)
            bc1_sb = cp.tile([64, 1], f32)
            bc2_sb = cp.tile([32, 1], f32)
            bc3_sb = cp.tile([16, 1], f32)
            bc4_sb = cp.tile([1, 1], f32)
            iota_sb = cp.tile([P, P], bf16)
            iop_sb = cp.tile([P, P], bf16)
            ident_sb = cp.tile([P, P], bf16)
            i64a_sb = cp.tile([G, G], f32)
            i64b_sb = cp.tile([G, G], f32)
            id64_sb = cp.tile([G, G], f32)
            zero_sb = cp.tile([P, 2048], f32)
            Spb_sb = cp.tile([P, NT * G], bf16)
            h1_sb = cp.tile([P, NSP], bf16)
            h1T_sb = cp.tile([P, NSP], bf16)
            h2e_sb = cp.tile([P, NT * 129], bf16)

            # ---- unpack blob ----
            for k in range(8):
                nc.sync.dma_start(out=esrc_sb[16 * k:16 * (k + 1), :],
                                  in_=bview("esrc"))
                nc.sync.dma_start(out=edst_sb[16 * k:16 * (k + 1), :],
                                  in_=bview("edst"))
            for dst_sb, name in [
                (gid8_sb, "gid"), (xq_sb, "xq"),
                (sisq_sb, "sisq"), (disq_sb, "disq"), (W1_sb, "W1"),
                (W2_sb, "W2"), (xscale_sb, "xscale"), (b1row_sb, "b1"),
                (b2row_sb, "b2"), (Wc1_sb, "Wc1"), (Wc2_sb, "Wc2"),
                (Wc3_sb, "Wc3"), (Wc4_sb, "Wc4"), (bc1_sb, "bc1"),
                (bc2_sb, "bc2"), (bc3_sb, "bc3"), (bc4_sb, "bc4"),
            ]:
                nc.sync.dma_start(out=dst_sb[:], in_=bview(name))

            # ---- derived constants / converts ----
            nc.gpsimd.iota(iota_sb[:], pattern=[[1, P]], base=0,
                           channel_multiplier=0,
                           allow_small_or_imprecise_dtypes=True)
            nc.gpsimd.iota(iop_sb[:], pattern=[[0, P]], base=0,
                           channel_multiplier=1,
                           allow_small_or_imprecise_dtypes=True)
            nc.vector.tensor_tensor(out=ident_sb[:], in0=iota_sb[:],
                                    in1=iop_sb[:], op=Alu.is_equal)
            nc.gpsimd.iota(i64a_sb[:], pattern=[[1, G]], base=0,
                           channel_multiplier=0,
                           allow_small_or_imprecise_dtypes=True)
            nc.gpsimd.iota(i64b_sb[:], pattern=[[0, G]], base=0,
                           channel_multiplier=1,
                           allow_small_or_imprecise_dtypes=True)
            nc.vector.tensor_tensor(out=id64_sb[:], in0=i64a_sb[:],
                                    in1=i64b_sb[:], op=Alu.is_equal)
            nc.vector.tensor_copy(gid_sb[:], gid8_sb[:])
            nc.vector.tensor_scalar(out=xT_sb[:], in0=xq_sb[:],
                                    scalar1=xscale_sb[:, 0:1], scalar2=None,
                                    op0=Alu.mult)
            nc.gpsimd.partition_broadcast(b1r_sb[:], b1row_sb[0:1, :])
            nc.gpsimd.partition_broadcast(b2r_sb[:], b2row_sb[0:1, :])
            nc.vector.memset(zero_sb[:], 0.0)
            nc.vector.memset(h2e_sb[:], 1.0)

            # ---- DRAM intermediates ----
            shard1 = dp.tile([NS, D], f32)
            table1 = dp.tile([N, D], f32, addr_space="Shared")
            shard2 = dp.tile([NS, D], f32)
            table2 = dp.tile([N, D], f32, addr_space="Shared")
            acc1 = dp.tile([NSP, D], f32)
            acc2 = dp.tile([NSP, D], f32)
            ar_in = dp.tile([G, 129], f32)
            ar_out = dp.tile([G, 129], f32, addr_space="Shared")

            # ================= helper: table build + allgather =============
            def build_table(hT_src_sb, W_sb, shard, table):
                with tc.tile_pool(name="psB", bufs=4, space="PSUM") as psB:
                    for i in range(NT):
                        ps = psB.tile([P, D], f32)
                        nc.tensor.matmul(
                            ps[:], lhsT=hT_src_sb[:, i * P:(i + 1) * P],
                            rhs=W_sb[:], start=True, stop=True)
                        sc_t = tp.tile([P, D], f32, tag="sct")
                        nc.vector.tensor_scalar(
                            out=sc_t[:], in0=ps[:],
                            scalar1=sisq_sb[:, i:i + 1], scalar2=None,
                            op0=Alu.mult)
                        lo = i * P
                        hi = min((i + 1) * P, NS)
                        if hi > lo:
                            nc.sync.dma_start(out=shard[lo:hi, :],
                                              in_=sc_t[:hi - lo, :])
                nc.gpsimd.collective_compute(
                    "AllGather", Alu.bypass, replica_groups=rg,
                    ins=[shard.opt()], outs=[table.opt()])

            # ================= helper: conv layer ==========================
            def conv_layer(table, acc, brd_sb, out_cb):
                """gather chunks from table, scatter-add into acc, then
                normalize per window; out_cb(w, pre_relu_tile)."""
                # zero the accumulator (incl. dump rows)
                accv = acc[:].rearrange("(p a) c -> p (a c)", p=P)
                col = 0
                while col < NSP:
                    nco = min(2048, NSP - col)
                    nc.sync.dma_start(out=accv[:, col:col + nco],
                                      in_=zero_sb[:, :nco])
                    col += nco
                for i in range(CT):
                    r0, r1 = (0, HALF) if i < CL else (HALF, N)
                    gb = gp.tile([P, TPC * P], f32, tag="gb")
                    gview = gb[:].rearrange("p (t c) -> p t c", c=P)
                    nc.gpsimd.dma_gather(
                        out_ap=gview,
                        in_ap=table[r0:r1, :],
                        idxs_ap=esrc_sb[:, i * ICOL:(i + 1) * ICOL],
                        num_idxs=OPI,
                        num_idxs_reg=OPI,
                        elem_size=D,
                        single_packet=False,
                    )
                    nc.gpsimd.dma_scatter_add(
                        out_ap=acc[:],
                        in_ap=gview,
                        idxs_ap=edst_sb[:, i * ICOL:(i + 1) * ICOL],
                        num_idxs=OPI,
                        num_idxs_reg=OPI,
                        elem_size=D,
                        single_packet=True,
                    )
                for w in range(NT):
                    aw = tp.tile([P, D], f32, tag="aw")
                    nc.sync.dma_start(out=aw[:],
                                      in_=acc[w * P:(w + 1) * P, :])
                    pre_t = tp.tile([P, D], f32, tag="pre")
                    nc.vector.scalar_tensor_tensor(
                        out=pre_t[:], in0=aw[:],
                        scalar=disq_sb[:, w:w + 1], in1=brd_sb[:],
                        op0=Alu.mult, op1=Alu.add)
                    out_cb(w, pre_t)

            # ================= Layer 1 =====================================
            build_table(xT_sb, W1_sb, shard1, table1)

            def l1_out(w, pre_t):
                nc.vector.tensor_scalar(
                    out=h1_sb[:, w * P:(w + 1) * P], in0=pre_t[:],
                    scalar1=0.0, scalar2=None, op0=Alu.max)

            conv_layer(table1, acc1, b1r_sb, l1_out)

            # transpose h1 tiles -> h1T
            with tc.tile_pool(name="psT", bufs=4, space="PSUM") as psT:
                for i in range(NT):
                    pst = psT.tile([P, P], bf16)
                    nc.tensor.transpose(pst[:], h1_sb[:, i * P:(i + 1) * P],
                                        ident_sb[:])
                    nc.vector.tensor_copy(h1T_sb[:, i * P:(i + 1) * P],
                                          pst[:])

            # ================= Layer 2 =====================================
            build_table(h1T_sb, W2_sb, shard2, table2)

            def l2_out(w, pre_t):
                nc.vector.tensor_scalar(
                    out=h2e_sb[:, w * 129:w * 129 + D], in0=pre_t[:],
                    scalar1=0.0, scalar2=None, op0=Alu.max)

            conv_layer(table2, acc2, b2r_sb, l2_out)

            # ================= Pooling + AllReduce =========================
            with tc.tile_pool(name="psP", bufs=2, space="PSUM") as psP:
                psp = psP.tile([G, 129], f32)
                Spb = Spb_sb
                nc.vector.tensor_tensor(
                    out=Spb[:].rearrange("p (t j) -> p t j", j=G),
                    in0=iota_sb[:, :G].rearrange("p (g j) -> p g j", g=1)
                        .to_broadcast([P, NT, G]),
                    in1=gid_sb[:].rearrange("p (c u) -> p c u", u=1)
                        .to_broadcast([P, NT, G]),
                    op=Alu.is_equal)
                for i in range(NT):
                    nc.tensor.matmul(psp[:], lhsT=Spb[:, i * G:(i + 1) * G],
                                     rhs=h2e_sb[:, i * 129:(i + 1) * 129],
                                     start=(i == 0), stop=(i == NT - 1))
                pool_sb = tp.tile([G, 129], f32, tag="pool")
                nc.vector.tensor_copy(pool_sb[:], psp[:])
                nc.sync.dma_start(out=ar_in[:], in_=pool_sb[:])

            nc.gpsimd.collective_compute(
                "AllReduce", Alu.add, replica_groups=rg,
                ins=[ar_in.opt()], outs=[ar_out.opt()])

            # ================= mean + MLP ==================================
            with tc.tile_pool(name="psM", bufs=1, space="PSUM") as psM:
                red_sb = tp.tile([G, 129], f32, tag="red")
                nc.sync.dma_start(out=red_sb[:], in_=ar_out[:])
                pcnt = tp.tile([G, 1], f32, tag="pcnt")
                nc.vector.tensor_scalar(out=pcnt[:], in0=red_sb[:, D:D + 1],
                                        scalar1=1.0, scalar2=None, op0=Alu.max)
                prcp = tp.tile([G, 1], f32, tag="prcp")
                nc.vector.reciprocal(prcp[:], pcnt[:])
                hg_sb = tp.tile([G, D], f32, tag="hg")
                nc.vector.tensor_scalar(out=hg_sb[:], in0=red_sb[:, 0:D],
                                        scalar1=prcp[:, :1], scalar2=None,
                                        op0=Alu.mult)
                ps_hgT = psM.tile([D, G], f32)
                nc.tensor.transpose(ps_hgT[:], hg_sb[:], id64_sb[:])
                hgT_sb = tp.tile([D, G], f32, tag="hgT")
                nc.vector.tensor_copy(hgT_sb[:], ps_hgT[:])

                ps1 = psM.tile([64, G], f32)
                nc.tensor.matmul(ps1[:], lhsT=Wc1_sb[:], rhs=hgT_sb[:],
                                 start=True, stop=True)
                o1_sb = tp.tile([64, G], f32, tag="o1")
                nc.scalar.activation(o1_sb[:], ps1[:], Act.Relu,
                                     bias=bc1_sb[:, :1])
                ps2 = psM.tile([32, G], f32)
                nc.tensor.matmul(ps2[:], lhsT=Wc2_sb[:], rhs=o1_sb[:],
                                 start=True, stop=True)
                o2_sb = tp.tile([32, G], f32, tag="o2")
                nc.scalar.activation(o2_sb[:], ps2[:], Act.Relu,
                                     bias=bc2_sb[:, :1])
                ps3 = psM.tile([16, G], f32)
                nc.tensor.matmul(ps3[:], lhsT=Wc3_sb[:], rhs=o2_sb[:],
                                 start=True, stop=True)
                o3_sb = tp.tile([16, G], f32, tag="o3")
                nc.scalar.activation(o3_sb[:], ps3[:], Act.Relu,
                                     bias=bc3_sb[:, :1])
                ps4 = psM.tile([1, G], f32)
                nc.tensor.matmul(ps4[:], lhsT=Wc4_sb[:], rhs=o3_sb[:],
                                 start=True, stop=True)
                out_sb = tp.tile([1, G], f32, tag="osb")
                nc.vector.tensor_scalar(out=out_sb[:], in0=ps4[:],
                                        scalar1=bc4_sb[:1, :1], scalar2=None,
                                        op0=Alu.add)
                nc.sync.dma_start(out=t_out[:], in_=out_sb[:])

    nc.compile()
    return nc


# ---------------------------------------------------------------------------
# Entry point
# ---------------------------------------------------------------------------

def kernel(x, src, dst, graph_id, num_graphs, W1, b1, W2, b2,
           Wc1, bc1, Wc2, bc2, Wc3, bc3, Wc4, bc4):
    import concourse.bass_utils as bass_utils

    assert int(num_graphs) == G

    shards, xscale, CL, CHH, OPI = _prep_blobs(x, src, dst, graph_id)
    blobs, _ = _pack_blobs(shards, xscale, CL, CHH, OPI, W1, b1, W2, b2,
                           Wc1, bc1, Wc2, bc2, Wc3, bc3, Wc4, bc4)

    in_maps = [dict(blob=b) for b in blobs]

    key = (CL, CHH, OPI)
    if key not in _PROGRAM_CACHE:
        _PROGRAM_CACHE[key] = _build_program(CL, CHH, OPI)
    nc = _PROGRAM_CACHE[key]

    global _last_in_maps
    _last_in_maps = in_maps

    res = bass_utils.run_bass_kernel_spmd(nc, in_maps, core_ids=list(range(C)))
    out = res.results[0]["out"]
    return np.asarray(out, dtype=np.float32).reshape(G, 1)


if __name__ == "__main__":
    import jax
    with jax.default_device(jax.devices("cpu")[0]):
        import reference
        inputs = reference.setup_inputs()
        inp = {k: (np.asarray(v) if hasattr(v, "shape") else v)
               for k, v in inputs.items()}
        expected = np.asarray(reference.reference(**inputs))
    got = kernel(**inp)
    err = np.abs(got - expected).max()
    rel = err / (np.abs(expected).max() + 1e-12)
    print("absmax err:", err, "rel:", rel)


# revision 24
# speedup vs baseline: 1.1238x; 1.1238x over previous
"""8-core Trainium2 Bass kernel for a 2-layer GCN + mean-pool + 4-layer MLP.

Strategy (graph/data parallel, per the sharding hint):
  - Nodes are partitioned into 8 contiguous ranges of 6250 (core c owns
    [c*6250, (c+1)*6250)).  Edges are bucketed by dst-owner.
  - Message tables (h @ W) * src_isqrt are built shard-wise (TensorE) and
    replicated with an AllGather.  Per-edge message rows are fetched from
    the table with indirect-DMA gathers (OPI rows x 512B per op) and
    accumulated into a per-node DRAM accumulator with indirect-DMA
    scatter-adds keyed by local dst (f32).  Each dst's edges are spread
    round-robin across ops on the host so no scatter op repeats a dst row
    (concurrent RMW on the same row within one op loses updates on HW).
    Normalization (D^-1/2 A D^-1/2) uses host-precomputed 1/sqrt(deg).
  - Per-graph pooled sums+counts [64,129] are AllReduce'd, and the small MLP
    runs replicated on every core.

  All per-core inputs are packed into a single uint8 blob (x quantized to
  int8, edge index streams int16) to minimize host->device transfer cost;
  the blob is sliced/bitcast on device.  Pad slots gather table row 0 and
  scatter into dump rows >= 6250 of the padded accumulator, keeping every
  DMA op full-size and static.
"""

import sys

import numpy as np

sys.path.insert(0, "/opt/trn_rl_repo")

import jax

# Persistent XLA compilation cache: the axon shim in bass_utils rebuilds the
# jit wrapper on every run_bass_kernel_spmd call, which re-invokes the full
# BIR->NEFF backend compile (~1.2s) unless the executable cache can serve it.
try:
    jax.config.update("jax_compilation_cache_dir", "/tmp/jax_cc_cache")
    jax.config.update("jax_persistent_cache_min_compile_time_secs", 0)
    jax.config.update("jax_persistent_cache_min_entry_size_bytes", -1)
except Exception:
    pass

import ml_dtypes

BF16 = ml_dtypes.bfloat16

N = 50000
E = 1600000
D = 128
G = 64
C = 8
NS = N // C            # 6250 nodes per core
P = 128
NT = (NS + P - 1) // P  # 49 node windows per core
NSP = NT * P            # 6272 (padded rows; >= NS are dump rows)
HALF = 25088


# ---------------------------------------------------------------------------
# Blob layout (shared between host packing and device program)
# ---------------------------------------------------------------------------

def _np_dt(mdt_name):
    return {
        "int8": np.int8, "uint8": np.uint8, "int16": np.int16,
        "bfloat16": BF16, "float32": np.float32,
    }[mdt_name]


def _blob_layout(CL, CHH, OPI):
    CT = CL + CHH
    specs = [
        ("xq", P, NSP, "int8"),
        ("esrc", 16, CT * (OPI // 16), "int16"),
        ("edst", 16, CT * (OPI // 16), "int16"),
        ("gid", P, NT, "int8"),
        ("sisq", P, NT, "float32"),
        ("disq", P, NT, "float32"),
        ("W1", D, D, "bfloat16"),
        ("W2", D, D, "bfloat16"),
        ("xscale", P, 1, "float32"),
        ("b1", 1, D, "float32"),
        ("b2", 1, D, "float32"),
        ("Wc1", D, 64, "float32"),
        ("Wc2", 64, 32, "float32"),
        ("Wc3", 32, 16, "float32"),
        ("Wc4", 16, 1, "float32"),
        ("bc1", 64, 1, "float32"),
        ("bc2", 32, 1, "float32"),
        ("bc3", 16, 1, "float32"),
        ("bc4", 1, 1, "float32"),
    ]
    layout = {}
    off = 0
    for name, r, c, dtn in specs:
        nb = r * c * np.dtype(_np_dt(dtn)).itemsize
        layout[name] = (off, r, c, dtn, nb)
        off += (nb + 511) // 512 * 512
    total = (off + 511) // 512 * 512
    return layout, total


# ---------------------------------------------------------------------------
# Host-side sharding prep
# ---------------------------------------------------------------------------

def _wrap16(vals):
    """vals [k*16] -> [16, k] wrapped (idx j at (j%16, j//16))."""
    return np.ascontiguousarray(vals.reshape(-1, 16).T)


def _rr_assign(d_local, nops):
    """Round-robin edges of each dst across ops so no op repeats a dst row
    (avoids duplicate-row RMW races within one scatter op).

    Returns (op, order) where edges (reordered by `order`) are grouped by
    op with per-op unique dst."""
    order = np.argsort(d_local, kind="stable")
    ds = d_local[order]
    first = np.searchsorted(ds, ds)
    rank = np.arange(ds.size) - first
    # `first` is the prefix sum of degrees, so starting each dst's
    # round-robin at first%nops packs the circular strip exactly: op loads
    # differ by at most 1 and within a dst ops stay unique (deg <= nops).
    op = (first + rank) % nops
    order2 = np.argsort(op, kind="stable")
    return op[order2], order[order2]


def _prep_blobs(x, src, dst, graph_id):
    src = np.asarray(src).astype(np.int64)
    dst = np.asarray(dst).astype(np.int64)
    x = np.asarray(x).astype(np.float32)
    graph_id = np.asarray(graph_id).astype(np.int64)

    out_deg = np.maximum(np.bincount(src, minlength=N), 1).astype(np.float64)
    in_deg = np.maximum(np.bincount(dst, minlength=N), 1).astype(np.float64)
    s_isq = (1.0 / np.sqrt(out_deg)).astype(np.float32)
    d_isq = (1.0 / np.sqrt(in_deg)).astype(np.float32)

    xscale = np.float32(np.abs(x).max() / 127.0)
    xq = np.clip(np.round(x / xscale), -127, 127).astype(np.int8)

    dst_owner = dst // NS
    lo_mask = src < HALF
    per_core = []
    CL = 0
    CHH = 0
    maxload = 0
    for c in range(C):
        me = dst_owner == c
        mlo = me & lo_mask
        mhi = me & ~lo_mask
        nlo = int(np.count_nonzero(mlo))
        nhi = int(np.count_nonzero(mhi))
        degl = int(np.bincount(dst[mlo] - c * NS, minlength=NS).max())
        degh = int(np.bincount(dst[mhi] - c * NS, minlength=NS).max())
        CL = max(CL, (nlo + 2047) // 2048, degl + 1)
        CHH = max(CHH, (nhi + 2047) // 2048, degh + 1)
        per_core.append((mlo, mhi))

    # per-op load with round-robin assignment; OPI = padded op capacity
    assigns = []
    for c in range(C):
        mlo, mhi = per_core[c]
        a_lo = _rr_assign((dst[mlo] - c * NS).astype(np.int64), CL)
        a_hi = _rr_assign((dst[mhi] - c * NS).astype(np.int64), CHH)
        for op, _ in (a_lo, a_hi):
            if op.size:
                maxload = max(maxload, int(np.bincount(op).max()))
        assigns.append((a_lo, a_hi))
    OPI = (maxload + 127) // 128 * 128

    shards = []
    for c in range(C):
        mlo, mhi = per_core[c]
        (op_lo, ord_lo), (op_hi, ord_hi) = assigns[c]
        parts_e = []
        parts_d = []
        for mask, nops, op, order, base in [
            (mlo, CL, op_lo, ord_lo, 0),
            (mhi, CHH, op_hi, ord_hi, HALF),
        ]:
            s_adj = (src[mask] - base).astype(np.int16)[order]
            d_loc = (dst[mask] - c * NS).astype(np.int16)[order]
            es = np.zeros(nops * OPI, dtype=np.int16)
            ds_ = np.full(nops * OPI, NS, dtype=np.int16)  # pad -> dump row
            pos = np.arange(op.size) - np.searchsorted(op, op)
            slot = op * OPI + pos
            assert pos.max(initial=0) < OPI
            es[slot] = s_adj
            ds_[slot] = d_loc
            parts_e.append(_wrap16(es))
            parts_d.append(_wrap16(ds_))
        esrc = np.concatenate(parts_e, axis=1)
        edst = np.concatenate(parts_d, axis=1)

        xqT = np.zeros((P, NSP), dtype=np.int8)
        xqT[:, :NS] = xq[c * NS:(c + 1) * NS].T
        gid = np.full((P, NT), -1, dtype=np.int8)
        gid.T.flat[:NS] = graph_id[c * NS:(c + 1) * NS].astype(np.int8)
        sisq = np.ones((P, NT), dtype=np.float32)
        sisq.T.flat[:NS] = s_isq[c * NS:(c + 1) * NS]
        disq = np.ones((P, NT), dtype=np.float32)
        disq.T.flat[:NS] = d_isq[c * NS:(c + 1) * NS]
        shards.append(dict(esrc=esrc, edst=edst, xq=xqT, gid=gid,
                           sisq=sisq, disq=disq))
    return shards, xscale, CL, CHH, OPI


def _pack_blobs(shards, xscale, CL, CHH, OPI, W1, b1, W2, b2,
                Wc1, bc1, Wc2, bc2, Wc3, bc3, Wc4, bc4):
    layout, total = _blob_layout(CL, CHH, OPI)
    common = dict(
        W1=np.asarray(W1).astype(BF16),
        W2=np.asarray(W2).astype(BF16),
        xscale=np.full((P, 1), xscale, dtype=np.float32),
        b1=np.asarray(b1, dtype=np.float32).reshape(1, D),
        b2=np.asarray(b2, dtype=np.float32).reshape(1, D),
        Wc1=np.asarray(Wc1, dtype=np.float32),
        Wc2=np.asarray(Wc2, dtype=np.float32),
        Wc3=np.asarray(Wc3, dtype=np.float32),
        Wc4=np.asarray(Wc4, dtype=np.float32),
        bc1=np.asarray(bc1, dtype=np.float32).reshape(64, 1),
        bc2=np.asarray(bc2, dtype=np.float32).reshape(32, 1),
        bc3=np.asarray(bc3, dtype=np.float32).reshape(16, 1),
        bc4=np.asarray(bc4, dtype=np.float32).reshape(1, 1),
    )
    blobs = []
    for sh in shards:
        blob = np.zeros((1, total), dtype=np.uint8)
        for name, (off, r, c, dtn, nb) in layout.items():
            arr = sh[name] if name in sh else common[name]
            a = np.ascontiguousarray(arr, dtype=_np_dt(dtn))
            assert a.shape == (r, c), (name, a.shape, (r, c))
            blob[0, off:off + nb] = a.view(np.uint8).reshape(-1)
        blobs.append(blob)
    return blobs, total


# ---------------------------------------------------------------------------
# Bass program
# ---------------------------------------------------------------------------

_PROGRAM_CACHE = {}


def _build_program(CL, CHH, OPI):
    CT = CL + CHH
    ICOL = OPI // 16             # idx cols per op
    TPC = OPI // P               # msg tiles per op
    layout, NBYTES = _blob_layout(CL, CHH, OPI)
    import concourse.bacc as bacc
    import concourse.mybir as mybir
    import concourse.tile as tile

    f32 = mybir.dt.float32
    bf16 = mybir.dt.bfloat16
    i8 = mybir.dt.int8
    i16 = mybir.dt.int16
    Alu = mybir.AluOpType
    Act = mybir.ActivationFunctionType
    MDT = {"int8": i8, "uint8": mybir.dt.uint8, "int16": i16,
           "bfloat16": bf16, "float32": f32}

    nc = bacc.Bacc("TRN2", target_bir_lowering=False, debug=False,
                   num_devices=C)

    t_blob = nc.dram_tensor("blob", [1, NBYTES], mybir.dt.uint8,
                            kind="ExternalInput")
    t_out = nc.dram_tensor("out", [1, G], f32, kind="ExternalOutput")

    def bview(name):
        off, r, c, dtn, nb = layout[name]
        return (t_blob[0:1, off:off + nb].bitcast(MDT[dtn])
                .rearrange("a (p c) -> (a p) c", p=r))

    rg = [list(range(C))]

    with tile.TileContext(nc) as tc:
        with (
            tc.tile_pool(name="const", bufs=1) as cp,
            tc.tile_pool(name="dram", bufs=1, space="DRAM") as dp,
            tc.tile_pool(name="gbuf", bufs=3) as gp,
            tc.tile_pool(name="tmp", bufs=6) as tp,
        ):
            # ---- persistent SBUF tensors ----
            esrc_sb = cp.tile([P, CT * ICOL], i16)
            edst_sb = cp.tile([P, CT * ICOL], i16)
            gid8_sb = cp.tile([P, NT], i8)
            gid_sb = cp.tile([P, NT], bf16)
            xq_sb = cp.tile([P, NSP], i8)
            xT_sb = cp.tile([P, NSP], bf16)
            sisq_sb = cp.tile([P, NT], f32)
            disq_sb = cp.tile([P, NT], f32)
            W1_sb = cp.tile([D, D], bf16)
            W2_sb = cp.tile([D, D], bf16)
            xscale_sb = cp.tile([P, 1], f32)
            b1row_sb = cp.tile([1, D], f32)
            b2row_sb = cp.tile([1, D], f32)
            b1r_sb = cp.tile([P, D], f32)
            b2r_sb = cp.tile([P, D], f32)
            Wc1_sb = cp.tile([D, 64], f32)
            Wc2_sb = cp.tile([64, 32], f32)
            Wc3_sb = cp.tile([32, 16], f32)
            Wc4_sb = cp.tile([16, 1], f32)
            bc1_sb = cp.tile([64, 1], f32)
            bc2_sb = cp.tile([32, 1], f32)
            bc3_sb = cp.tile([16, 1], f32)
            bc4_sb = cp.tile([1, 1], f32)
            iota_sb = cp.tile([P, P], bf16)
            iop_sb = cp.tile([P, P], bf16)
            ident_sb = cp.tile([P, P], bf16)
            i64a_sb = cp.tile([G, G], f32)
            i64b_sb = cp.tile([G, G], f32)
            id64_sb = cp.tile([G, G], f32)
            zero_sb = cp.tile([P, 2048], f32)
            Spb_sb = cp.tile([P, NT * G], bf16)
            accall_sb = cp.tile([P, NT * P], f32)
            h1_sb = cp.tile([P, NSP], bf16)
            h1T_sb = cp.tile([P, NSP], bf16)
            h2e_sb = cp.tile([P, NT * 129], bf16)

            # ---- unpack blob ----
            for k in range(8):
                nc.sync.dma_start(out=esrc_sb[16 * k:16 * (k + 1), :],
                                  in_=bview("esrc"))
                nc.sync.dma_start(out=edst_sb[16 * k:16 * (k + 1), :],
                                  in_=bview("edst"))
            for dst_sb, name in [
                (gid8_sb, "gid"), (xq_sb, "xq"),
                (sisq_sb, "sisq"), (disq_sb, "disq"), (W1_sb, "W1"),
                (W2_sb, "W2"), (xscale_sb, "xscale"), (b1row_sb, "b1"),
                (b2row_sb, "b2"), (Wc1_sb, "Wc1"), (Wc2_sb, "Wc2"),
                (Wc3_sb, "Wc3"), (Wc4_sb, "Wc4"), (bc1_sb, "bc1"),
                (bc2_sb, "bc2"), (bc3_sb, "bc3"), (bc4_sb, "bc4"),
            ]:
                nc.sync.dma_start(out=dst_sb[:], in_=bview(name))

            # ---- derived constants / converts ----
            nc.gpsimd.iota(iota_sb[:], pattern=[[1, P]], base=0,
                           channel_multiplier=0,
                           allow_small_or_imprecise_dtypes=True)
            nc.gpsimd.iota(iop_sb[:], pattern=[[0, P]], base=0,
                           channel_multiplier=1,
                           allow_small_or_imprecise_dtypes=True)
            nc.vector.tensor_tensor(out=ident_sb[:], in0=iota_sb[:],
                                    in1=iop_sb[:], op=Alu.is_equal)
            nc.gpsimd.iota(i64a_sb[:], pattern=[[1, G]], base=0,
                           channel_multiplier=0,
                           allow_small_or_imprecise_dtypes=True)
            nc.gpsimd.iota(i64b_sb[:], pattern=[[0, G]], base=0,
                           channel_multiplier=1,
                           allow_small_or_imprecise_dtypes=True)
            nc.vector.tensor_tensor(out=id64_sb[:], in0=i64a_sb[:],
                                    in1=i64b_sb[:], op=Alu.is_equal)
            nc.vector.tensor_copy(gid_sb[:], gid8_sb[:])
            nc.vector.tensor_scalar(out=xT_sb[:], in0=xq_sb[:],
                                    scalar1=xscale_sb[:, 0:1], scalar2=None,
                                    op0=Alu.mult)
            nc.gpsimd.partition_broadcast(b1r_sb[:], b1row_sb[0:1, :])
            nc.gpsimd.partition_broadcast(b2r_sb[:], b2row_sb[0:1, :])
            nc.vector.memset(zero_sb[:], 0.0)
            nc.vector.memset(h2e_sb[:], 1.0)

            # ---- DRAM intermediates ----
            shard1 = dp.tile([NS, D], f32)
            table1 = dp.tile([N, D], f32, addr_space="Shared")
            shard2 = dp.tile([NS, D], f32)
            table2 = dp.tile([N, D], f32, addr_space="Shared")
            acc1 = dp.tile([NSP, D], f32)
            acc2 = dp.tile([NSP, D], f32)
            ar_in = dp.tile([G, 129], f32)
            ar_out = dp.tile([G, 129], f32, addr_space="Shared")

            # ================= helper: table build + allgather =============
            def build_table(hT_src_sb, W_sb, shard, table):
                with tc.tile_pool(name="psB", bufs=4, space="PSUM") as psB:
                    for i in range(NT):
                        ps = psB.tile([P, D], f32)
                        nc.tensor.matmul(
                            ps[:], lhsT=hT_src_sb[:, i * P:(i + 1) * P],
                            rhs=W_sb[:], start=True, stop=True)
                        sc_t = tp.tile([P, D], f32, tag="sct")
                        nc.vector.tensor_scalar(
                            out=sc_t[:], in0=ps[:],
                            scalar1=sisq_sb[:, i:i + 1], scalar2=None,
                            op0=Alu.mult)
                        lo = i * P
                        hi = min((i + 1) * P, NS)
                        if hi > lo:
                            nc.sync.dma_start(out=shard[lo:hi, :],
                                              in_=sc_t[:hi - lo, :])
                nc.gpsimd.collective_compute(
                    "AllGather", Alu.bypass, replica_groups=rg,
                    ins=[shard.opt()], outs=[table.opt()])

            # ================= helper: conv layer ==========================
            def conv_layer(table, acc, brd_sb, out_view):
                """gather chunks from table, scatter-add into acc, then
                batch-normalize the whole shard; relu into out_view
                ([P, NT, D] AP)."""
                # zero the accumulator (incl. dump rows)
                accv = acc[:].rearrange("(p a) c -> p (a c)", p=P)
                col = 0
                while col < NSP:
                    nco = min(2048, NSP - col)
                    nc.sync.dma_start(out=accv[:, col:col + nco],
                                      in_=zero_sb[:, :nco])
                    col += nco
                for i in range(CT):
                    r0, r1 = (0, HALF) if i < CL else (HALF, N)
                    gb = gp.tile([P, TPC * P], f32, tag="gb")
                    gview = gb[:].rearrange("p (t c) -> p t c", c=P)
                    nc.gpsimd.dma_gather(
                        out_ap=gview,
                        in_ap=table[r0:r1, :],
                        idxs_ap=esrc_sb[:, i * ICOL:(i + 1) * ICOL],
                        num_idxs=OPI,
                        num_idxs_reg=OPI,
                        elem_size=D,
                        single_packet=False,
                    )
                    nc.gpsimd.dma_scatter_add(
                        out_ap=acc[:],
                        in_ap=gview,
                        idxs_ap=edst_sb[:, i * ICOL:(i + 1) * ICOL],
                        num_idxs=OPI,
                        num_idxs_reg=OPI,
                        elem_size=D,
                        single_packet=True,
                    )
                # load whole accumulator as [node%128, window, feat]
                nc.sync.dma_start(
                    out=accall_sb[:].rearrange("p (w c) -> p w c", c=D),
                    in_=acc[:].rearrange("(w p) c -> p w c", p=P))
                av = accall_sb[:].rearrange("p (w c) -> p w c", c=D)
                nc.vector.tensor_tensor(
                    out=av, in0=av,
                    in1=disq_sb[:].rearrange("p (w u) -> p w u", u=1)
                        .to_broadcast([P, NT, D]),
                    op=Alu.mult)
                nc.vector.tensor_tensor(
                    out=av, in0=av,
                    in1=brd_sb[:].rearrange("p (g c) -> p g c", g=1)
                        .to_broadcast([P, NT, D]),
                    op=Alu.add)
                nc.vector.tensor_scalar(
                    out=out_view, in0=av,
                    scalar1=0.0, scalar2=None, op0=Alu.max)

            # ================= Layer 1 =====================================
            build_table(xT_sb, W1_sb, shard1, table1)

            conv_layer(table1, acc1, b1r_sb,
                       h1_sb[:].rearrange("p (w c) -> p w c", c=P))

            # transpose h1 tiles -> h1T
            with tc.tile_pool(name="psT", bufs=4, space="PSUM") as psT:
                for i in range(NT):
                    pst = psT.tile([P, P], bf16)
                    nc.tensor.transpose(pst[:], h1_sb[:, i * P:(i + 1) * P],
                                        ident_sb[:])
                    nc.vector.tensor_copy(h1T_sb[:, i * P:(i + 1) * P],
                                          pst[:])

            # ================= Layer 2 =====================================
            build_table(h1T_sb, W2_sb, shard2, table2)

            conv_layer(table2, acc2, b2r_sb,
                       h2e_sb[:].rearrange("p (w c) -> p w c", c=129)
                       [:, :, 0:D])

            # ================= Pooling + AllReduce =========================
            with tc.tile_pool(name="psP", bufs=2, space="PSUM") as psP:
                psp = psP.tile([G, 129], f32)
                Spb = Spb_sb
                nc.vector.tensor_tensor(
                    out=Spb[:].rearrange("p (t j) -> p t j", j=G),
                    in0=iota_sb[:, :G].rearrange("p (g j) -> p g j", g=1)
                        .to_broadcast([P, NT, G]),
                    in1=gid_sb[:].rearrange("p (c u) -> p c u", u=1)
                        .to_broadcast([P, NT, G]),
                    op=Alu.is_equal)
                for i in range(NT):
                    nc.tensor.matmul(psp[:], lhsT=Spb[:, i * G:(i + 1) * G],
                                     rhs=h2e_sb[:, i * 129:(i + 1) * 129],
                                     start=(i == 0), stop=(i == NT - 1))
                pool_sb = tp.tile([G, 129], f32, tag="pool")
                nc.vector.tensor_copy(pool_sb[:], psp[:])
                nc.sync.dma_start(out=ar_in[:], in_=pool_sb[:])

            nc.gpsimd.collective_compute(
                "AllReduce", Alu.add, replica_groups=rg,
                ins=[ar_in.opt()], outs=[ar_out.opt()])

            # ================= mean + MLP ==================================
            with tc.tile_pool(name="psM", bufs=1, space="PSUM") as psM:
                red_sb = tp.tile([G, 129], f32, tag="red")
                nc.sync.dma_start(out=red_sb[:], in_=ar_out[:])
                pcnt = tp.tile([G, 1], f32, tag="pcnt")
                nc.vector.tensor_scalar(out=pcnt[:], in0=red_sb[:, D:D + 1],
                                        scalar1=1.0, scalar2=None, op0=Alu.max)
                prcp = tp.tile([G, 1], f32, tag="prcp")
                nc.vector.reciprocal(prcp[:], pcnt[:])
                hg_sb = tp.tile([G, D], f32, tag="hg")
                nc.vector.tensor_scalar(out=hg_sb[:], in0=red_sb[:, 0:D],
                                        scalar1=prcp[:, :1], scalar2=None,
                                        op0=Alu.mult)
                ps_hgT = psM.tile([D, G], f32)
                nc.tensor.transpose(ps_hgT[:], hg_sb[:], id64_sb[:])
                hgT_sb = tp.tile([D, G], f32, tag="hgT")
                nc.vector.tensor_copy(hgT_sb[:], ps_hgT[:])

                ps1 = psM.tile([64, G], f32)
                nc.tensor.matmul(ps1[:], lhsT=Wc1_sb[:], rhs=hgT_sb[:],
                                 start=True, stop=True)
                o1_sb = tp.tile([64, G], f32, tag="o1")
                nc.scalar.activation(o1_sb[:], ps1[:], Act.Relu,
                                     bias=bc1_sb[:, :1])
                ps2 = psM.tile([32, G], f32)
                nc.tensor.matmul(ps2[:], lhsT=Wc2_sb[:], rhs=o1_sb[:],
                                 start=True, stop=True)
                o2_sb = tp.tile([32, G], f32, tag="o2")
                nc.scalar.activation(o2_sb[:], ps2[:], Act.Relu,
                                     bias=bc2_sb[:, :1])
                ps3 = psM.tile([16, G], f32)
                nc.tensor.matmul(ps3[:], lhsT=Wc3_sb[:], rhs=o2_sb[:],
                                 start=True, stop=True)
                o3_sb = tp.tile([16, G], f32, tag="o3")
                nc.scalar.activation(o3_sb[:], ps3[:], Act.Relu,
                                     bias=bc3_sb[:, :1])
                ps4 = psM.tile([1, G], f32)
                nc.tensor.matmul(ps4[:], lhsT=Wc4_sb[:], rhs=o3_sb[:],
                                 start=True, stop=True)
                out_sb = tp.tile([1, G], f32, tag="osb")
                nc.vector.tensor_scalar(out=out_sb[:], in0=ps4[:],
                                        scalar1=bc4_sb[:1, :1], scalar2=None,
                                        op0=Alu.add)
                nc.sync.dma_start(out=t_out[:], in_=out_sb[:])

    nc.compile()
    return nc


# ---------------------------------------------------------------------------
# Entry point
# ---------------------------------------------------------------------------

def kernel(x, src, dst, graph_id, num_graphs, W1, b1, W2, b2,
           Wc1, bc1, Wc2, bc2, Wc3, bc3, Wc4, bc4):
    import concourse.bass_utils as bass_utils

    assert int(num_graphs) == G

    shards, xscale, CL, CHH, OPI = _prep_blobs(x, src, dst, graph_id)
    blobs, _ = _pack_blobs(shards, xscale, CL, CHH, OPI, W1, b1, W2, b2,
                           Wc1, bc1, Wc2, bc2, Wc3, bc3, Wc4, bc4)

    in_maps = [dict(blob=b) for b in blobs]

    key = (CL, CHH, OPI)
    if key not in _PROGRAM_CACHE:
        _PROGRAM_CACHE[key] = _build_program(CL, CHH, OPI)
    nc = _PROGRAM_CACHE[key]

    global _last_in_maps
    _last_in_maps = in_maps

    res = bass_utils.run_bass_kernel_spmd(nc, in_maps, core_ids=list(range(C)))
    out = res.results[0]["out"]
    return np.asarray(out, dtype=np.float32).reshape(G, 1)


if __name__ == "__main__":
    import jax
    with jax.default_device(jax.devices("cpu")[0]):
        import reference
        inputs = reference.setup_inputs()
        inp = {k: (np.asarray(v) if hasattr(v, "shape") else v)
               for k, v in inputs.items()}
        expected = np.asarray(reference.reference(**inputs))
    got = kernel(**inp)
    err = np.abs(got - expected).max()
    rel = err / (np.abs(expected).max() + 1e-12)
    print("absmax err:", err, "rel:", rel)


# revision 27
# speedup vs baseline: 1.1516x; 1.0247x over previous
"""8-core Trainium2 Bass kernel for a 2-layer GCN + mean-pool + 4-layer MLP.

Strategy (graph/data parallel, per the sharding hint):
  - Nodes are partitioned into 8 contiguous ranges of 6250 (core c owns
    [c*6250, (c+1)*6250)).  Edges are bucketed by dst-owner.
  - Message tables (h @ W) * src_isqrt are built shard-wise (TensorE) and
    replicated with an AllGather.  Per-edge message rows are fetched from
    the table with indirect-DMA gathers (OPI rows x 512B per op) and
    accumulated into a per-node DRAM accumulator with indirect-DMA
    scatter-adds keyed by local dst (f32).  Each dst's edges are spread
    round-robin across ops on the host so no scatter op repeats a dst row
    (concurrent RMW on the same row within one op loses updates on HW).
    Normalization (D^-1/2 A D^-1/2) uses host-precomputed 1/sqrt(deg).
  - Per-graph pooled sums+counts [64,129] are AllReduce'd, and the small MLP
    runs replicated on every core.

  All per-core inputs are packed into a single uint8 blob (x quantized to
  int8, edge index streams int16) to minimize host->device transfer cost;
  the blob is sliced/bitcast on device.  Pad slots gather table row 0 and
  scatter into dump rows >= 6250 of the padded accumulator, keeping every
  DMA op full-size and static.
"""

import sys

import numpy as np

sys.path.insert(0, "/opt/trn_rl_repo")

import jax

# Persistent XLA compilation cache: the axon shim in bass_utils rebuilds the
# jit wrapper on every run_bass_kernel_spmd call, which re-invokes the full
# BIR->NEFF backend compile (~1.2s) unless the executable cache can serve it.
try:
    jax.config.update("jax_compilation_cache_dir", "/tmp/jax_cc_cache")
    jax.config.update("jax_persistent_cache_min_compile_time_secs", 0)
    jax.config.update("jax_persistent_cache_min_entry_size_bytes", -1)
except Exception:
    pass

import ml_dtypes

BF16 = ml_dtypes.bfloat16

N = 50000
E = 1600000
D = 128
G = 64
C = 8
NS = N // C            # 6250 nodes per core
P = 128
NT = (NS + P - 1) // P  # 49 node windows per core
NSP = NT * P            # 6272 (padded rows; >= NS are dump rows)
HALF = 25088


# ---------------------------------------------------------------------------
# Blob layout (shared between host packing and device program)
# ---------------------------------------------------------------------------

def _np_dt(mdt_name):
    return {
        "int8": np.int8, "uint8": np.uint8, "int16": np.int16,
        "bfloat16": BF16, "float32": np.float32,
    }[mdt_name]


def _blob_layout(CL, CHH, OPI):
    CT = CL + CHH
    specs = [
        ("xq", P, NSP, "int8"),
        ("esrc", 16, CT * (OPI // 16), "int16"),
        ("edst", 16, CT * (OPI // 16), "int16"),
        ("gid", P, NT, "int8"),
        ("sisq", P, NT, "float32"),
        ("disq", P, NT, "float32"),
        ("W1", D, D, "bfloat16"),
        ("W2", D, D, "bfloat16"),
        ("xscale", P, 1, "float32"),
        ("b1", 1, D, "float32"),
        ("b2", 1, D, "float32"),
        ("Wc1", D, 64, "float32"),
        ("Wc2", 64, 32, "float32"),
        ("Wc3", 32, 16, "float32"),
        ("Wc4", 16, 1, "float32"),
        ("bc1", 64, 1, "float32"),
        ("bc2", 32, 1, "float32"),
        ("bc3", 16, 1, "float32"),
        ("bc4", 1, 1, "float32"),
    ]
    layout = {}
    off = 0
    for name, r, c, dtn in specs:
        nb = r * c * np.dtype(_np_dt(dtn)).itemsize
        layout[name] = (off, r, c, dtn, nb)
        off += (nb + 511) // 512 * 512
    total = (off + 511) // 512 * 512
    return layout, total


# ---------------------------------------------------------------------------
# Host-side sharding prep
# ---------------------------------------------------------------------------

def _wrap16(vals):
    """vals [k*16] -> [16, k] wrapped (idx j at (j%16, j//16))."""
    return np.ascontiguousarray(vals.reshape(-1, 16).T)


def _rr_assign(d_local, nops):
    """Round-robin edges of each dst across ops so no op repeats a dst row
    (avoids duplicate-row RMW races within one scatter op).

    Returns (op, order) where edges (reordered by `order`) are grouped by
    op with per-op unique dst."""
    order = np.argsort(d_local, kind="stable")
    ds = d_local[order]
    first = np.searchsorted(ds, ds)
    rank = np.arange(ds.size) - first
    # `first` is the prefix sum of degrees, so starting each dst's
    # round-robin at first%nops packs the circular strip exactly: op loads
    # differ by at most 1 and within a dst ops stay unique (deg <= nops).
    op = (first + rank) % nops
    order2 = np.argsort(op, kind="stable")
    return op[order2], order[order2]


def _prep_blobs(x, src, dst, graph_id):
    src = np.asarray(src).astype(np.int64)
    dst = np.asarray(dst).astype(np.int64)
    x = np.asarray(x).astype(np.float32)
    graph_id = np.asarray(graph_id).astype(np.int64)

    out_deg = np.maximum(np.bincount(src, minlength=N), 1).astype(np.float64)
    in_deg = np.maximum(np.bincount(dst, minlength=N), 1).astype(np.float64)
    s_isq = (1.0 / np.sqrt(out_deg)).astype(np.float32)
    d_isq = (1.0 / np.sqrt(in_deg)).astype(np.float32)

    xscale = np.float32(np.abs(x).max() / 127.0)
    xq = np.clip(np.round(x / xscale), -127, 127).astype(np.int8)

    dst_owner = dst // NS
    lo_mask = src < HALF
    per_core = []
    CL = 0
    CHH = 0
    maxload = 0
    for c in range(C):
        me = dst_owner == c
        mlo = me & lo_mask
        mhi = me & ~lo_mask
        nlo = int(np.count_nonzero(mlo))
        nhi = int(np.count_nonzero(mhi))
        degl = int(np.bincount(dst[mlo] - c * NS, minlength=NS).max())
        degh = int(np.bincount(dst[mhi] - c * NS, minlength=NS).max())
        CL = max(CL, (nlo + 2047) // 2048, degl + 1)
        CHH = max(CHH, (nhi + 2047) // 2048, degh + 1)
        per_core.append((mlo, mhi))

    # per-op load with round-robin assignment; OPI = padded op capacity
    assigns = []
    for c in range(C):
        mlo, mhi = per_core[c]
        a_lo = _rr_assign((dst[mlo] - c * NS).astype(np.int64), CL)
        a_hi = _rr_assign((dst[mhi] - c * NS).astype(np.int64), CHH)
        for op, _ in (a_lo, a_hi):
            if op.size:
                maxload = max(maxload, int(np.bincount(op).max()))
        assigns.append((a_lo, a_hi))
    OPI = (maxload + 127) // 128 * 128

    shards = []
    for c in range(C):
        mlo, mhi = per_core[c]
        (op_lo, ord_lo), (op_hi, ord_hi) = assigns[c]
        parts_e = []
        parts_d = []
        for mask, nops, op, order, base in [
            (mlo, CL, op_lo, ord_lo, 0),
            (mhi, CHH, op_hi, ord_hi, HALF),
        ]:
            s_adj = (src[mask] - base).astype(np.int16)[order]
            d_loc = (dst[mask] - c * NS).astype(np.int16)[order]
            es = np.zeros(nops * OPI, dtype=np.int16)
            ds_ = np.full(nops * OPI, NS, dtype=np.int16)  # pad -> dump row
            pos = np.arange(op.size) - np.searchsorted(op, op)
            slot = op * OPI + pos
            assert pos.max(initial=0) < OPI
            es[slot] = s_adj
            ds_[slot] = d_loc
            parts_e.append(_wrap16(es))
            parts_d.append(_wrap16(ds_))
        esrc = np.concatenate(parts_e, axis=1)
        edst = np.concatenate(parts_d, axis=1)

        xqT = np.zeros((P, NSP), dtype=np.int8)
        xqT[:, :NS] = xq[c * NS:(c + 1) * NS].T
        gid = np.full((P, NT), -1, dtype=np.int8)
        gid.T.flat[:NS] = graph_id[c * NS:(c + 1) * NS].astype(np.int8)
        sisq = np.ones((P, NT), dtype=np.float32)
        sisq.T.flat[:NS] = s_isq[c * NS:(c + 1) * NS]
        disq = np.ones((P, NT), dtype=np.float32)
        disq.T.flat[:NS] = d_isq[c * NS:(c + 1) * NS]
        shards.append(dict(esrc=esrc, edst=edst, xq=xqT, gid=gid,
                           sisq=sisq, disq=disq))
    return shards, xscale, CL, CHH, OPI


def _pack_blobs(shards, xscale, CL, CHH, OPI, W1, b1, W2, b2,
                Wc1, bc1, Wc2, bc2, Wc3, bc3, Wc4, bc4):
    layout, total = _blob_layout(CL, CHH, OPI)
    common = dict(
        W1=np.asarray(W1).astype(BF16),
        W2=np.asarray(W2).astype(BF16),
        xscale=np.full((P, 1), xscale, dtype=np.float32),
        b1=np.asarray(b1, dtype=np.float32).reshape(1, D),
        b2=np.asarray(b2, dtype=np.float32).reshape(1, D),
        Wc1=np.asarray(Wc1, dtype=np.float32),
        Wc2=np.asarray(Wc2, dtype=np.float32),
        Wc3=np.asarray(Wc3, dtype=np.float32),
        Wc4=np.asarray(Wc4, dtype=np.float32),
        bc1=np.asarray(bc1, dtype=np.float32).reshape(64, 1),
        bc2=np.asarray(bc2, dtype=np.float32).reshape(32, 1),
        bc3=np.asarray(bc3, dtype=np.float32).reshape(16, 1),
        bc4=np.asarray(bc4, dtype=np.float32).reshape(1, 1),
    )
    blobs = []
    for sh in shards:
        blob = np.zeros((1, total), dtype=np.uint8)
        for name, (off, r, c, dtn, nb) in layout.items():
            arr = sh[name] if name in sh else common[name]
            a = np.ascontiguousarray(arr, dtype=_np_dt(dtn))
            assert a.shape == (r, c), (name, a.shape, (r, c))
            blob[0, off:off + nb] = a.view(np.uint8).reshape(-1)
        blobs.append(blob)
    return blobs, total


# ---------------------------------------------------------------------------
# Bass program
# ---------------------------------------------------------------------------

_PROGRAM_CACHE = {}


def _build_program(CL, CHH, OPI):
    CT = CL + CHH
    ICOL = OPI // 16             # idx cols per op
    TPC = OPI // P               # msg tiles per op
    layout, NBYTES = _blob_layout(CL, CHH, OPI)
    import concourse.bacc as bacc
    import concourse.mybir as mybir
    import concourse.tile as tile

    f32 = mybir.dt.float32
    bf16 = mybir.dt.bfloat16
    i8 = mybir.dt.int8
    i16 = mybir.dt.int16
    Alu = mybir.AluOpType
    Act = mybir.ActivationFunctionType
    MDT = {"int8": i8, "uint8": mybir.dt.uint8, "int16": i16,
           "bfloat16": bf16, "float32": f32}

    nc = bacc.Bacc("TRN2", target_bir_lowering=False, debug=False,
                   num_devices=C)

    t_blob = nc.dram_tensor("blob", [1, NBYTES], mybir.dt.uint8,
                            kind="ExternalInput")
    t_out = nc.dram_tensor("out", [1, G], f32, kind="ExternalOutput")

    def bview(name):
        off, r, c, dtn, nb = layout[name]
        return (t_blob[0:1, off:off + nb].bitcast(MDT[dtn])
                .rearrange("a (p c) -> (a p) c", p=r))

    rg = [list(range(C))]

    with tile.TileContext(nc) as tc:
        with (
            tc.tile_pool(name="const", bufs=1) as cp,
            tc.tile_pool(name="dram", bufs=1, space="DRAM") as dp,
            tc.tile_pool(name="gbuf", bufs=2) as gp,
            tc.tile_pool(name="tmp", bufs=6) as tp,
        ):
            # ---- persistent SBUF tensors ----
            esrc_sb = cp.tile([P, CT * ICOL], i16)
            edst_sb = cp.tile([P, CT * ICOL], i16)
            gid8_sb = cp.tile([P, NT], i8)
            gid_sb = cp.tile([P, NT], bf16)
            xq_sb = cp.tile([P, NSP], i8)
            xT_sb = cp.tile([P, NSP], bf16)
            sisq_sb = cp.tile([P, NT], f32)
            disq_sb = cp.tile([P, NT], f32)
            W1_sb = cp.tile([D, D], bf16)
            W2_sb = cp.tile([D, D], bf16)
            xscale_sb = cp.tile([P, 1], f32)
            b1row_sb = cp.tile([1, D], f32)
            b2row_sb = cp.tile([1, D], f32)
            b1r_sb = cp.tile([P, D], f32)
            b2r_sb = cp.tile([P, D], f32)
            Wc1_sb = cp.tile([D, 64], f32)
            Wc2_sb = cp.tile([64, 32], f32)
            Wc3_sb = cp.tile([32, 16], f32)
            Wc4_sb = cp.tile([16, 1], f32)
            bc1_sb = cp.tile([64, 1], f32)
            bc2_sb = cp.tile([32, 1], f32)
            bc3_sb = cp.tile([16, 1], f32)
            bc4_sb = cp.tile([1, 1], f32)
            iota_sb = cp.tile([P, P], bf16)
            iop_sb = cp.tile([P, P], bf16)
            ident_sb = cp.tile([P, P], bf16)
            i64a_sb = cp.tile([G, G], f32)
            i64b_sb = cp.tile([G, G], f32)
            id64_sb = cp.tile([G, G], f32)
            zero_sb = cp.tile([P, 1024], f32)
            Spb_sb = cp.tile([P, NT * G], bf16)
            accall_sb = cp.tile([P, NT * P], f32)
            h1_sb = cp.tile([P, NSP], bf16)
            h1T_sb = cp.tile([P, NSP], bf16)
            h2e_sb = cp.tile([P, NT * 129], bf16)

            # ---- unpack blob ----
            for k in range(8):
                nc.sync.dma_start(out=esrc_sb[16 * k:16 * (k + 1), :],
                                  in_=bview("esrc"))
                nc.sync.dma_start(out=edst_sb[16 * k:16 * (k + 1), :],
                                  in_=bview("edst"))
            for dst_sb, name in [
                (gid8_sb, "gid"), (xq_sb, "xq"),
                (sisq_sb, "sisq"), (disq_sb, "disq"), (W1_sb, "W1"),
                (W2_sb, "W2"), (xscale_sb, "xscale"), (b1row_sb, "b1"),
                (b2row_sb, "b2"), (Wc1_sb, "Wc1"), (Wc2_sb, "Wc2"),
                (Wc3_sb, "Wc3"), (Wc4_sb, "Wc4"), (bc1_sb, "bc1"),
                (bc2_sb, "bc2"), (bc3_sb, "bc3"), (bc4_sb, "bc4"),
            ]:
                nc.sync.dma_start(out=dst_sb[:], in_=bview(name))

            # ---- derived constants / converts ----
            nc.gpsimd.iota(iota_sb[:], pattern=[[1, P]], base=0,
                           channel_multiplier=0,
                           allow_small_or_imprecise_dtypes=True)
            nc.gpsimd.iota(iop_sb[:], pattern=[[0, P]], base=0,
                           channel_multiplier=1,
                           allow_small_or_imprecise_dtypes=True)
            nc.vector.tensor_tensor(out=ident_sb[:], in0=iota_sb[:],
                                    in1=iop_sb[:], op=Alu.is_equal)
            nc.gpsimd.iota(i64a_sb[:], pattern=[[1, G]], base=0,
                           channel_multiplier=0,
                           allow_small_or_imprecise_dtypes=True)
            nc.gpsimd.iota(i64b_sb[:], pattern=[[0, G]], base=0,
                           channel_multiplier=1,
                           allow_small_or_imprecise_dtypes=True)
            nc.vector.tensor_tensor(out=id64_sb[:], in0=i64a_sb[:],
                                    in1=i64b_sb[:], op=Alu.is_equal)
            nc.vector.tensor_copy(gid_sb[:], gid8_sb[:])
            nc.vector.tensor_scalar(out=xT_sb[:], in0=xq_sb[:],
                                    scalar1=xscale_sb[:, 0:1], scalar2=None,
                                    op0=Alu.mult)
            nc.gpsimd.partition_broadcast(b1r_sb[:], b1row_sb[0:1, :])
            nc.gpsimd.partition_broadcast(b2r_sb[:], b2row_sb[0:1, :])
            nc.vector.memset(zero_sb[:], 0.0)
            nc.vector.memset(h2e_sb[:], 1.0)

            # ---- DRAM intermediates ----
            shard1 = dp.tile([NS, D], f32)
            table1 = dp.tile([N, D], f32, addr_space="Shared")
            shard2 = dp.tile([NS, D], f32)
            table2 = dp.tile([N, D], f32, addr_space="Shared")
            acc1 = dp.tile([NSP, D], f32)
            acc2 = dp.tile([NSP, D], f32)
            ar_in = dp.tile([G, 129], f32)
            ar_out = dp.tile([G, 129], f32, addr_space="Shared")

            # ================= helper: table build + allgather =============
            def build_table(hT_src_sb, W_sb, shard, table):
                with tc.tile_pool(name="psB", bufs=4, space="PSUM") as psB:
                    for i in range(NT):
                        ps = psB.tile([P, D], f32)
                        nc.tensor.matmul(
                            ps[:], lhsT=hT_src_sb[:, i * P:(i + 1) * P],
                            rhs=W_sb[:], start=True, stop=True)
                        sc_t = tp.tile([P, D], f32, tag="sct")
                        nc.vector.tensor_scalar(
                            out=sc_t[:], in0=ps[:],
                            scalar1=sisq_sb[:, i:i + 1], scalar2=None,
                            op0=Alu.mult)
                        lo = i * P
                        hi = min((i + 1) * P, NS)
                        if hi > lo:
                            nc.sync.dma_start(out=shard[lo:hi, :],
                                              in_=sc_t[:hi - lo, :])
                nc.gpsimd.collective_compute(
                    "AllGather", Alu.bypass, replica_groups=rg,
                    ins=[shard.opt()], outs=[table.opt()])

            # ================= helper: conv layer ==========================
            def conv_layer(table, acc, brd_sb, out_view):
                """gather chunks from table, scatter-add into acc, then
                batch-normalize the whole shard; relu into out_view
                ([P, NT, D] AP)."""
                # zero the accumulator (incl. dump rows)
                accv = acc[:].rearrange("(p a) c -> p (a c)", p=P)
                col = 0
                while col < NSP:
                    nco = min(1024, NSP - col)
                    nc.sync.dma_start(out=accv[:, col:col + nco],
                                      in_=zero_sb[:, :nco])
                    col += nco
                # paired gathers: one 2*OPI-row gather feeds two OPI-row
                # scatters (adjacent ops' wrapped idx columns are contiguous)
                for base, nops, r0, r1 in [(0, CL, 0, HALF),
                                           (CL, CHH, HALF, N)]:
                    i = 0
                    while i < nops:
                        pair = 2 if i + 1 < nops else 1
                        gb = gp.tile([P, 2 * TPC * P], f32, tag="gb")
                        gview = gb[:].rearrange("p (t c) -> p t c", c=P)
                        o0 = base + i
                        nc.gpsimd.dma_gather(
                            out_ap=gview[:, :pair * TPC, :],
                            in_ap=table[r0:r1, :],
                            idxs_ap=esrc_sb[:, o0 * ICOL:
                                            (o0 + pair) * ICOL],
                            num_idxs=pair * OPI,
                            num_idxs_reg=pair * OPI,
                            elem_size=D,
                            single_packet=False,
                        )
                        for k in range(pair):
                            nc.gpsimd.dma_scatter_add(
                                out_ap=acc[:],
                                in_ap=gview[:, k * TPC:(k + 1) * TPC, :],
                                idxs_ap=edst_sb[:, (o0 + k) * ICOL:
                                                (o0 + k + 1) * ICOL],
                                num_idxs=OPI,
                                num_idxs_reg=OPI,
                                elem_size=D,
                                single_packet=True,
                            )
                        i += pair
                # load whole accumulator as [node%128, window, feat]
                nc.sync.dma_start(
                    out=accall_sb[:].rearrange("p (w c) -> p w c", c=D),
                    in_=acc[:].rearrange("(w p) c -> p w c", p=P))
                av = accall_sb[:].rearrange("p (w c) -> p w c", c=D)
                nc.vector.tensor_tensor(
                    out=av, in0=av,
                    in1=disq_sb[:].rearrange("p (w u) -> p w u", u=1)
                        .to_broadcast([P, NT, D]),
                    op=Alu.mult)
                nc.vector.tensor_tensor(
                    out=av, in0=av,
                    in1=brd_sb[:].rearrange("p (g c) -> p g c", g=1)
                        .to_broadcast([P, NT, D]),
                    op=Alu.add)
                nc.vector.tensor_scalar(
                    out=out_view, in0=av,
                    scalar1=0.0, scalar2=None, op0=Alu.max)

            # ================= Layer 1 =====================================
            build_table(xT_sb, W1_sb, shard1, table1)

            conv_layer(table1, acc1, b1r_sb,
                       h1_sb[:].rearrange("p (w c) -> p w c", c=P))

            # transpose h1 tiles -> h1T
            with tc.tile_pool(name="psT", bufs=4, space="PSUM") as psT:
                for i in range(NT):
                    pst = psT.tile([P, P], bf16)
                    nc.tensor.transpose(pst[:], h1_sb[:, i * P:(i + 1) * P],
                                        ident_sb[:])
                    nc.vector.tensor_copy(h1T_sb[:, i * P:(i + 1) * P],
                                          pst[:])

            # ================= Layer 2 =====================================
            build_table(h1T_sb, W2_sb, shard2, table2)

            conv_layer(table2, acc2, b2r_sb,
                       h2e_sb[:].rearrange("p (w c) -> p w c", c=129)
                       [:, :, 0:D])

            # ================= Pooling + AllReduce =========================
            with tc.tile_pool(name="psP", bufs=2, space="PSUM") as psP:
                psp = psP.tile([G, 129], f32)
                Spb = Spb_sb
                nc.vector.tensor_tensor(
                    out=Spb[:].rearrange("p (t j) -> p t j", j=G),
                    in0=iota_sb[:, :G].rearrange("p (g j) -> p g j", g=1)
                        .to_broadcast([P, NT, G]),
                    in1=gid_sb[:].rearrange("p (c u) -> p c u", u=1)
                        .to_broadcast([P, NT, G]),
                    op=Alu.is_equal)
                for i in range(NT):
                    nc.tensor.matmul(psp[:], lhsT=Spb[:, i * G:(i + 1) * G],
                                     rhs=h2e_sb[:, i * 129:(i + 1) * 129],
                                     start=(i == 0), stop=(i == NT - 1))
                pool_sb = tp.tile([G, 129], f32, tag="pool")
                nc.vector.tensor_copy(pool_sb[:], psp[:])
                nc.sync.dma_start(out=ar_in[:], in_=pool_sb[:])

            nc.gpsimd.collective_compute(
                "AllReduce", Alu.add, replica_groups=rg,
                ins=[ar_in.opt()], outs=[ar_out.opt()])

            # ================= mean + MLP ==================================
            with tc.tile_pool(name="psM", bufs=1, space="PSUM") as psM:
                red_sb = tp.tile([G, 129], f32, tag="red")
                nc.sync.dma_start(out=red_sb[:], in_=ar_out[:])
                pcnt = tp.tile([G, 1], f32, tag="pcnt")
                nc.vector.tensor_scalar(out=pcnt[:], in0=red_sb[:, D:D + 1],
                                        scalar1=1.0, scalar2=None, op0=Alu.max)
                prcp = tp.tile([G, 1], f32, tag="prcp")
                nc.vector.reciprocal(prcp[:], pcnt[:])
                hg_sb = tp.tile([G, D], f32, tag="hg")
                nc.vector.tensor_scalar(out=hg_sb[:], in0=red_sb[:, 0:D],
                                        scalar1=prcp[:, :1], scalar2=None,
                                        op0=Alu.mult)
                ps_hgT = psM.tile([D, G], f32)
                nc.tensor.transpose(ps_hgT[:], hg_sb[:], id64_sb[:])
                hgT_sb = tp.tile([D, G], f32, tag="hgT")
                nc.vector.tensor_copy(hgT_sb[:], ps_hgT[:])

                ps1 = psM.tile([64, G], f32)
                nc.tensor.matmul(ps1[:], lhsT=Wc1_sb[:], rhs=hgT_sb[:],
                                 start=True, stop=True)
                o1_sb = tp.tile([64, G], f32, tag="o1")
                nc.scalar.activation(o1_sb[:], ps1[:], Act.Relu,
                                     bias=bc1_sb[:, :1])
                ps2 = psM.tile([32, G], f32)
                nc.tensor.matmul(ps2[:], lhsT=Wc2_sb[:], rhs=o1_sb[:],
                                 start=True, stop=True)
                o2_sb = tp.tile([32, G], f32, tag="o2")
                nc.scalar.activation(o2_sb[:], ps2[:], Act.Relu,
                                     bias=bc2_sb[:, :1])
                ps3 = psM.tile([16, G], f32)
                nc.tensor.matmul(ps3[:], lhsT=Wc3_sb[:], rhs=o2_sb[:],
                                 start=True, stop=True)
                o3_sb = tp.tile([16, G], f32, tag="o3")
                nc.scalar.activation(o3_sb[:], ps3[:], Act.Relu,
                                     bias=bc3_sb[:, :1])
                ps4 = psM.tile([1, G], f32)
                nc.tensor.matmul(ps4[:], lhsT=Wc4_sb[:], rhs=o3_sb[:],
                                 start=True, stop=True)
                out_sb = tp.tile([1, G], f32, tag="osb")
                nc.vector.tensor_scalar(out=out_sb[:], in0=ps4[:],
                                        scalar1=bc4_sb[:1, :1], scalar2=None,
                                        op0=Alu.add)
                nc.sync.dma_start(out=t_out[:], in_=out_sb[:])

    nc.compile()
    return nc


# ---------------------------------------------------------------------------
# Entry point
# ---------------------------------------------------------------------------

def kernel(x, src, dst, graph_id, num_graphs, W1, b1, W2, b2,
           Wc1, bc1, Wc2, bc2, Wc3, bc3, Wc4, bc4):
    import concourse.bass_utils as bass_utils

    assert int(num_graphs) == G

    shards, xscale, CL, CHH, OPI = _prep_blobs(x, src, dst, graph_id)
    blobs, _ = _pack_blobs(shards, xscale, CL, CHH, OPI, W1, b1, W2, b2,
                           Wc1, bc1, Wc2, bc2, Wc3, bc3, Wc4, bc4)

    in_maps = [dict(blob=b) for b in blobs]

    key = (CL, CHH, OPI)
    if key not in _PROGRAM_CACHE:
        _PROGRAM_CACHE[key] = _build_program(CL, CHH, OPI)
    nc = _PROGRAM_CACHE[key]

    global _last_in_maps
    _last_in_maps = in_maps

    res = bass_utils.run_bass_kernel_spmd(nc, in_maps, core_ids=list(range(C)))
    out = res.results[0]["out"]
    return np.asarray(out, dtype=np.float32).reshape(G, 1)


if __name__ == "__main__":
    import jax
    with jax.default_device(jax.devices("cpu")[0]):
        import reference
        inputs = reference.setup_inputs()
        inp = {k: (np.asarray(v) if hasattr(v, "shape") else v)
               for k, v in inputs.items()}
        expected = np.asarray(reference.reference(**inputs))
    got = kernel(**inp)
    err = np.abs(got - expected).max()
    rel = err / (np.abs(expected).max() + 1e-12)
    print("absmax err:", err, "rel:", rel)
